# revision 3
# baseline (speedup 1.0000x reference)
"""Edge-MLP GNN message passing kernel for Trainium2 (8 NeuronCores).

Per edge e = (u, v): out[e] = sigmoid(relu(|x[u] - x[v]| @ W1 + b1) @ W2 + b2)

The baseline is Pool-engine-bound: SWDGE descriptor generation for the
two dma_gathers costs ~8.35 ns/index (1.37 ms of the 1.5 ms kernel).
v2 removes one side from Pool entirely:

  - Host bin-packs nodes into 84 bins (<=128 nodes, <=1024 edge-slots
    per bin by u-endpoint) and lays edges out bin-contiguously with
    static 1024-slot bin regions (~7% padding).
  - u-side "gather" on the PE: per bin, one-hot matmul
    psum_u = xb_bin^T @ S (xb_bin = bin's node rows as lhsT, S = host
    -built one-hot fp16). 2 static 512-col matmuls per bin.
  - v-side on Pool dma_gather as before (86016 idx).
  - d = |psum_u - g_v| via DVE subtract + ACT abs.
  - W1/relu as baseline (two 512-groups packed in PSUM halves).
  - W2 restructured as row-matmuls: out_row[1, 512] = w2^T @ h, rows
    accumulated across groups into two persistent [128, 512] PSUM
    tiles; two ACT sigmoids at the end.
"""

import os
import sys

for _p in ("/opt/trn_rl_repo", "/root/.axon_site/_ro/trn_rl_repo"):
    if os.path.isdir(_p) and _p not in sys.path:
        sys.path.insert(0, _p)

import numpy as np

import concourse.bacc as bacc
import concourse.mybir as mybir
from concourse.mybir import AluOpType
from concourse.tile import TileContext
from concourse.bass_utils import run_bass_kernel_spmd

N_NODES = 10000
N_EDGES = 640000
D_FEAT = 128
HID = 64
N_CORES = 8
E_CORE = N_EDGES // N_CORES  # 80000

B_BINS = 79
CAP = 1024                   # edge slots per bin (static)
SLOTS = B_BINS * CAP         # 86016 padded edge slots per core
SUPER = 8                    # bins per (full) super-chunk
# taper the tail: last chunks small so the final gather DMA + MLP tail
# after the last desc-gen is short
CHUNK_BINS = [8] * 9 + [6, 1]
assert sum(CHUNK_BINS) == B_BINS
OUT_COLS = SLOTS // 128      # 672 output columns (edge pos = col*128 + p)

f16 = mybir.dt.float16
f32 = mybir.dt.float32
i16 = mybir.dt.int16

_NC_CACHE = None


def _build_nc():
    nc = bacc.Bacc(
        "TRN2", target_bir_lowering=False, dynamic_dma_scratch_size=32768,
    )

    xb_d = nc.dram_tensor("xb", [128, B_BINS * D_FEAT], f16, kind="ExternalInput")
    sg_d = nc.dram_tensor("sg", [128, SLOTS], f16, kind="ExternalInput")
    vidx_d = nc.dram_tensor("vidx", [128, SLOTS // 16], i16, kind="ExternalInput")
    x16_d = nc.dram_tensor("x16", [N_NODES, D_FEAT], f16, kind="ExternalInput")
    w1_d = nc.dram_tensor("w1", [D_FEAT, HID], f16, kind="ExternalInput")
    w2_d = nc.dram_tensor("w2", [128, 1], f16, kind="ExternalInput")
    b1_d = nc.dram_tensor("b1", [128, 1], f32, kind="ExternalInput")
    b2_d = nc.dram_tensor("b2", [128, 1], f32, kind="ExternalInput")
    out_d = nc.dram_tensor("out", [128, OUT_COLS], f32, kind="ExternalOutput")

    with TileContext(nc) as tc:
        with (
            tc.tile_pool(name="const", bufs=1) as cpool,
            tc.tile_pool(name="sone", bufs=2) as spool,
            tc.tile_pool(name="gath", bufs=2) as gpool,
            tc.tile_pool(name="diff", bufs=2) as dpool,
            tc.tile_pool(name="hid", bufs=4) as hpool,
            tc.tile_pool(name="outp", bufs=1) as opool,
            tc.tile_pool(name="psu", bufs=2, space="PSUM") as pupool,
            tc.tile_pool(name="psm", bufs=3, space="PSUM") as pmpool,
            tc.tile_pool(name="psr", bufs=2, space="PSUM") as prpool,
        ):
            xb = cpool.tile([128, B_BINS * D_FEAT], f16, tag="xb")
            vidx = cpool.tile([128, SLOTS // 16], i16, tag="vidx")
            w1 = cpool.tile([D_FEAT, HID], f16, tag="w1")
            w2 = cpool.tile([128, 1], f16, tag="w2")
            b1 = cpool.tile([128, 1], f32, tag="b1")
            b2 = cpool.tile([128, 1], f32, tag="b2")
            out_sb = opool.tile([128, OUT_COLS], f32, tag="osb")

            nc.sync.dma_start(vidx[:], vidx_d[:])
            nc.sync.dma_start(xb[:], xb_d[:])
            nc.sync.dma_start(w1[:], w1_d[:])
            nc.sync.dma_start(w2[:], w2_d[:])
            nc.sync.dma_start(b1[:], b1_d[:])
            nc.sync.dma_start(b2[:], b2_d[:])

            e0 = 0
            col0 = 0
            for s, nbins in enumerate(CHUNK_BINS):
                cols = nbins * CAP

                S_sb = spool.tile([128, cols], f16, tag="S")
                nc.sync.dma_start(S_sb[:], sg_d[:, e0 : e0 + cols])

                g1 = gpool.tile([128, cols], f16, tag="g1")
                nc.gpsimd.dma_gather(
                    g1[:].rearrange("p (a c) -> p a c", a=1),
                    x16_d[:],
                    vidx[:, e0 // 16 : (e0 + cols) // 16],
                    cols,
                    cols,
                    elem_size=D_FEAT,
                    transpose=True,
                    single_packet=False,
                )

                d = dpool.tile([128, cols], f16, tag="d")
                # u-side gather on PE + subtract, 512 cols at a time
                for t in range(cols // 512):
                    b = e0 // CAP + t // 2  # global bin
                    pu = pupool.tile([128, 512], f32, tag="pu")
                    nc.tensor.matmul(
                        pu[:],
                        xb[:, b * D_FEAT : (b + 1) * D_FEAT],
                        S_sb[:, t * 512 : (t + 1) * 512],
                        start=True, stop=True,
                    )
                    nc.vector.tensor_tensor(
                        d[:, t * 512 : (t + 1) * 512],
                        pu[:],
                        g1[:, t * 512 : (t + 1) * 512],
                        AluOpType.subtract,
                    )
                nc.scalar.activation(
                    d[:], d[:], mybir.ActivationFunctionType.Abs,
                )

                # MLP per 1024-edge group (two 512-groups in PSUM halves)
                ncols_s = cols // 128
                p2 = prpool.tile([128, ncols_s], f32, tag="p2")
                colc = 0
                for gl in range(nbins):
                    g0c = gl * CAP
                    pm = pmpool.tile([128, 512], f32, tag="pm")
                    nc.tensor.matmul(
                        pm[0:HID, :], w1[:], d[:, g0c : g0c + 512],
                        start=True, stop=True,
                    )
                    nc.tensor.matmul(
                        pm[HID:128, :], w1[:], d[:, g0c + 512 : g0c + 1024],
                        start=True, stop=True,
                    )
                    h = hpool.tile([128, 512], f16, tag="h")
                    nc.vector.tensor_scalar(
                        h[:], pm[:], b1[:], 0.0, AluOpType.add, AluOpType.max,
                    )
                    for j in range(4):
                        nc.tensor.matmul(
                            p2[:, colc : colc + 1],
                            h[0:HID, j * 128 : (j + 1) * 128],
                            w2[0:HID, :],
                            start=True, stop=True,
                        )
                        colc += 1
                    for j in range(4):
                        nc.tensor.matmul(
                            p2[:, colc : colc + 1],
                            h[HID:128, j * 128 : (j + 1) * 128],
                            w2[HID:128, :],
                            start=True, stop=True,
                        )
                        colc += 1
                nc.scalar.activation(
                    out_sb[:, col0 : col0 + ncols_s], p2[:, 0:ncols_s],
                    mybir.ActivationFunctionType.Sigmoid,
                    bias=b2[:], scale=1.0,
                )
                e0 += cols
                col0 += ncols_s

            nc.sync.dma_start(out_d[:], out_sb[:])

    nc.finalize()
    return nc


def _get_nc():
    global _NC_CACHE
    if _NC_CACHE is None:
        _NC_CACHE = _build_nc()
    return _NC_CACHE


def _interleave_idx(a):
    n = a.shape[0]
    m = a.reshape(n // 16, 16).T.astype(np.int16)
    return np.tile(m, (8, 1))


def _pack_core(u, v):
    """Bin-pack one core's edges. Returns (xb_sel, S, vidx_padded, pos)."""
    e = u.shape[0]
    deg = np.bincount(u, minlength=N_NODES)
    order_nodes = np.argsort(-deg, kind="stable")
    bin_load = np.zeros(B_BINS, dtype=np.int64)
    bin_count = np.zeros(B_BINS, dtype=np.int64)
    node_bin = np.empty(N_NODES, dtype=np.int32)
    node_slot = np.empty(N_NODES, dtype=np.int32)
    # greedy min-load assignment (vectorized-ish: argmin per node)
    for n in order_nodes:
        cand = np.where(bin_count < 128)[0]
        b = cand[np.argmin(bin_load[cand])]
        node_bin[n] = b
        node_slot[n] = bin_count[b]
        bin_count[b] += 1
        bin_load[b] += deg[n]
    assert bin_load.max() <= CAP, f"bin overflow: {bin_load.max()}"

    # edge slots: edges sorted by bin of u, laid into bin regions
    ebin = node_bin[u]
    eorder = np.argsort(ebin, kind="stable")
    # position within bin region
    pos = np.empty(e, dtype=np.int64)
    counts = np.bincount(ebin, minlength=B_BINS)
    starts = np.arange(B_BINS) * CAP
    offs = np.concatenate([[0], np.cumsum(counts)])[:-1]
    rank = np.empty(e, dtype=np.int64)
    rank[eorder] = np.arange(e)
    pos = starts[ebin] + (rank - offs[ebin])

    S = np.zeros((128, SLOTS), dtype=np.float16)
    S[node_slot[u], pos] = np.float16(1.0)

    vfull = np.zeros(SLOTS, dtype=np.int16)
    vfull[pos] = v.astype(np.int16)

    xb_sel = np.zeros((128, B_BINS * D_FEAT), dtype=np.float16)
    return node_bin, node_slot, S, vfull, pos, xb_sel


def prep_in_maps(x, indices, W1, b1, W2, b2):
    x32 = np.asarray(x, dtype=np.float32)
    x16 = np.ascontiguousarray(x32).astype(np.float16)
    idx = np.asarray(indices)
    w1 = x32 if False else np.asarray(W1, dtype=np.float32).astype(np.float16)
    w2c = np.asarray(W2, dtype=np.float32).astype(np.float16).reshape(HID, 1)
    w2s = np.concatenate([w2c, w2c], axis=0)
    b1c = np.asarray(b1, dtype=np.float32).reshape(HID, 1)
    b1s = np.concatenate([b1c, b1c], axis=0)
    b2s = np.full((128, 1), np.asarray(b2, dtype=np.float32).reshape(-1)[0],
                  dtype=np.float32)

    in_maps = []
    pos_all = []
    for c in range(N_CORES):
        sl = slice(c * E_CORE, (c + 1) * E_CORE)
        u = idx[0, sl].astype(np.int64)
        v = idx[1, sl].astype(np.int64)
        node_bin, node_slot, S, vfull, pos, xb_sel = _pack_core(u, v)
        # xb: bin-arranged node rows (lhsT layout [128 slots, bins*128 feats])
        for b in range(B_BINS):
            nodes = np.where(node_bin == b)[0]
            slots = node_slot[nodes]
            xb_sel[slots, b * D_FEAT : (b + 1) * D_FEAT] = x16[nodes]
        pos_all.append(pos)
        in_maps.append({
            "xb": xb_sel,
            "sg": S,
            "vidx": _interleave_idx(vfull),
            "x16": x16,
            "w1": w1,
            "w2": w2s,
            "b1": b1s,
            "b2": b2s,
        })
    return in_maps, pos_all


def run_hw(x, indices, W1, b1, W2, b2, trace=False, **kw):
    nc = _get_nc()
    in_maps, pos_all = prep_in_maps(x, indices, W1, b1, W2, b2)
    res = run_bass_kernel_spmd(
        nc, in_maps, core_ids=list(range(N_CORES)), trace=trace, **kw
    )
    outs = []
    for c in range(N_CORES):
        o = np.asarray(res.results[c]["out"])  # [128, 672]
        padded = o.T.reshape(-1)  # padded pos = col*128 + p
        outs.append(padded[pos_all[c]])
    return np.concatenate(outs), res


def kernel(x, indices, W1, b1, W2, b2):
    out, _ = run_hw(x, indices, W1, b1, W2, b2, trace=False)
    return out.astype(np.float32)
